# revision 2
# baseline (speedup 1.0000x reference)
"""Trainium2 Bass kernel for nn_MultiHeadAttention_87694642250090.

Multi-head attention (B=2, T=2048, D=1024, H=16 heads, dk=64) returning
(out [B,T,D], attn [B,H,T,T]).

Sharding: 8 cores = 2 batches x 4 head-groups (4 heads each), SPMD (one
program, per-core data).  Inputs are passed pre-transposed ([D, T]) so every
matmul has its contraction dim on partitions.

Per-core pipeline (per q-row-block i of 128, per head h):
  scores   = QT_h^T-slice @ KT_h            (PE, natural [tq, tk] layout)
  scoresT  = twin matmuls                   (PE, [tk, tq] layout for PV)
  (+ causal: triangular -1e30 const added to the diagonal 128x128 block)
  em  = exp(scores * 1/8)   with accum_out => per-row softmax denominator
  emT = exp(scoresT * 1/8)
  attn_out = em * (1/sum)                   (DVE, per-partition scalar)
  ctx = sum_kt emT_kt^T @ V_kt              (PE, contraction on tk)
  out_partial = ctxT @ Wo_slice             (PE)

Causal specialization skips all fully-masked upper blocks; the attn output is
written packed (lower-triangular block rows only) and scattered into zeros on
the host.
"""

import sys

if "/opt/trn_rl_repo" not in sys.path:
    sys.path.insert(0, "/opt/trn_rl_repo")

import numpy as np

B, T, D, H = 2, 2048, 1024, 16
DK = D // H          # 64 head dim
HPC = H // 4         # 4 heads per core
DKC = HPC * DK       # 256 dk per core
P = 128
KS = D // P          # 8 contraction subtiles for projections
S2 = DKC // P        # 2 dk-tiles of 128
NEG = -1.0e30

_built = {}


def _pack_size(nt):
    return P * P * (nt * (nt + 1) // 2)


def _build(t_len, causal):
    """Build the per-core SPMD Bass program. Returns (nc, input names meta)."""
    import concourse.bacc as bacc
    import concourse.mybir as mybir
    import concourse.tile as tile

    f32 = mybir.dt.float32
    u8 = mybir.dt.uint8
    NT = t_len // P           # q/k row blocks
    NC4 = (NT + 3) // 4       # 512-col chunks
    AF = mybir.ActivationFunctionType
    ALU = mybir.AluOpType
    AX = mybir.AxisListType

    nc = bacc.Bacc("TRN2", target_bir_lowering=False, debug=False)

    # ---- DRAM I/O ----
    qT = nc.dram_tensor("qT", [D, t_len], f32, kind="ExternalInput")
    kT = nc.dram_tensor("kT", [D, t_len], f32, kind="ExternalInput")
    vT = nc.dram_tensor("vT", [D, t_len], f32, kind="ExternalInput")
    wq = nc.dram_tensor("wq", [D, DKC], f32, kind="ExternalInput")
    wk = nc.dram_tensor("wk", [D, DKC], f32, kind="ExternalInput")
    wv = nc.dram_tensor("wv", [D, DKC], f32, kind="ExternalInput")
    wo = nc.dram_tensor("wo", [DKC, D], f32, kind="ExternalInput")
    bq2 = nc.dram_tensor("bq2", [P, S2], f32, kind="ExternalInput")
    bk2 = nc.dram_tensor("bk2", [P, S2], f32, kind="ExternalInput")
    bv1 = nc.dram_tensor("bv1", [1, DKC], f32, kind="ExternalInput")
    ident = nc.dram_tensor("ident", [P, P], f32, kind="ExternalInput")
    if causal:
        tri = nc.dram_tensor("tri", [P, P], f32, kind="ExternalInput")
        triT = nc.dram_tensor("triT", [P, P], f32, kind="ExternalInput")
        pack = _pack_size(NT)
    else:
        mrows = nc.dram_tensor("mrows", [t_len, t_len], u8, kind="ExternalInput")
        mcolsT = nc.dram_tensor("mcolsT", [t_len, t_len], u8, kind="ExternalInput")
        pack = NT * P * t_len

    out_p = nc.dram_tensor("out_p", [t_len, D], f32, kind="ExternalOutput")
    attn_p = nc.dram_tensor("attn_p", [HPC, pack], f32, kind="ExternalOutput")

    with tile.TileContext(nc) as tc:
        with (
            tc.tile_pool(name="const", bufs=1) as cpool,
            tc.tile_pool(name="inchunk", bufs=2) as inpool,
            tc.tile_pool(name="em", bufs=2) as empool,
            tc.tile_pool(name="emt", bufs=2) as emtpool,
            tc.tile_pool(name="ao", bufs=2) as aopool,
            tc.tile_pool(name="small", bufs=4) as smpool,
            tc.tile_pool(name="ctx", bufs=2) as ctxpool,
            tc.tile_pool(name="outp", bufs=2) as outpool,
            tc.tile_pool(name="mask", bufs=2) as mkpool,
            tc.tile_pool(name="psum", bufs=2, space="PSUM") as pspool,
        ):
            # ---- persistent SBUF ----
            wq_sb = cpool.tile([P, KS, DKC], f32)
            wk_sb = cpool.tile([P, KS, DKC], f32)
            wv_sb = cpool.tile([P, KS, DKC], f32)
            wo_sb = cpool.tile([P, S2, D], f32)
            bq_sb = cpool.tile([P, S2], f32)
            bk_sb = cpool.tile([P, S2], f32)
            bv_sb = cpool.tile([1, DKC], f32)
            id_sb = cpool.tile([P, P], f32)
            ones_sb = cpool.tile([1, P], f32)
            QT_sb = cpool.tile([P, S2, t_len], f32)
            KT_sb = cpool.tile([P, S2, t_len], f32)
            V_sb = cpool.tile([P, NT, DKC], f32)
            if causal:
                tri_sb = cpool.tile([P, P], f32)
                triT_sb = cpool.tile([P, P], f32)

            nc.sync.dma_start(wq_sb[:], wq.rearrange("(ks p) m -> p ks m", p=P))
            nc.sync.dma_start(wk_sb[:], wk.rearrange("(ks p) m -> p ks m", p=P))
            nc.sync.dma_start(wv_sb[:], wv.rearrange("(ks p) m -> p ks m", p=P))
            nc.sync.dma_start(wo_sb[:], wo.rearrange("(s p) n -> p s n", p=P))
            nc.sync.dma_start(bq_sb[:], bq2[:])
            nc.sync.dma_start(bk_sb[:], bk2[:])
            nc.sync.dma_start(bv_sb[:], bv1[:])
            nc.sync.dma_start(id_sb[:], ident[:])
            if causal:
                nc.sync.dma_start(tri_sb[:], tri[:])
                nc.sync.dma_start(triT_sb[:], triT[:])
            nc.vector.memset(ones_sb[:], 1.0)

            qT3 = qT.rearrange("(ks p) t -> p ks t", p=P)
            kT3 = kT.rearrange("(ks p) t -> p ks t", p=P)
            vT3 = vT.rearrange("(ks p) t -> p ks t", p=P)

            # ---- phase A: projections (128-col t-chunks to bound SBUF) ----
            for kt in range(NT):
                tsl = slice(kt * P, (kt + 1) * P)
                qc = inpool.tile([P, KS, P], f32, tag="qc")
                kc_ = inpool.tile([P, KS, P], f32, tag="kc")
                vc = inpool.tile([P, KS, P], f32, tag="vc")
                nc.sync.dma_start(qc[:], qT3[:, :, tsl])
                nc.sync.dma_start(kc_[:], kT3[:, :, tsl])
                nc.sync.dma_start(vc[:], vT3[:, :, tsl])

                for s in range(S2):
                    pq = pspool.tile([P, 512], f32, tag="S")
                    pk = pspool.tile([P, 512], f32, tag="ST")
                    for ks in range(KS):
                        nc.tensor.matmul(
                            pq[:, :P],
                            lhsT=wq_sb[:, ks, s * P:(s + 1) * P],
                            rhs=qc[:, ks, :],
                            start=(ks == 0), stop=(ks == KS - 1),
                        )
                    for ks in range(KS):
                        nc.tensor.matmul(
                            pk[:, :P],
                            lhsT=wk_sb[:, ks, s * P:(s + 1) * P],
                            rhs=kc_[:, ks, :],
                            start=(ks == 0), stop=(ks == KS - 1),
                        )
                    nc.scalar.activation(
                        QT_sb[:, s, tsl], pq[:, :P], AF.Identity,
                        bias=bq_sb[:, s:s + 1],
                    )
                    nc.scalar.activation(
                        KT_sb[:, s, tsl], pk[:, :P], AF.Identity,
                        bias=bk_sb[:, s:s + 1],
                    )

                # V natural [t, dk] for this 128-row block
                pv = pspool.tile([P, DKC], f32, tag="V")
                for ks in range(KS):
                    nc.tensor.matmul(
                        pv[:],
                        lhsT=vc[:, ks, :],
                        rhs=wv_sb[:, ks, :],
                        start=(ks == 0), stop=False,
                    )
                nc.tensor.matmul(
                    pv[:], lhsT=ones_sb[0:1, :], rhs=bv_sb[0:1, :],
                    start=False, stop=True,
                )
                nc.vector.tensor_copy(V_sb[:, kt, :], pv[:])

            # ---- phase B: attention ----
            for i in range(NT):
                nkt_i = (i + 1) if causal else NT
                qsl = slice(i * P, (i + 1) * P)

                if not causal:
                    mrow = mkpool.tile([P, t_len], u8, tag="mrow")
                    nc.sync.dma_start(mrow[:], mrows[qsl, :])
                    mcol = mkpool.tile([P, NT, P], u8, tag="mcol")
                    nc.sync.dma_start(
                        mcol[:, :nkt_i, :],
                        mcolsT.rearrange("(kt p) q -> p kt q", p=P)[:, :nkt_i, qsl],
                    )

                ctx_sb = ctxpool.tile([P, DKC], f32, tag="ctxs")

                for h in range(HPC):
                    hb = (h % 2) * DK
                    hs = h // 2
                    nkc = (nkt_i + 3) // 4
                    lhsQ = QT_sb[hb:hb + DK, hs, qsl]

                    em = empool.tile([P, t_len], f32, tag="em")
                    emT = emtpool.tile([P, NT, P], f32, tag="emT")
                    sig = smpool.tile([P, NC4], f32, tag="sig")

                    for kc in range(nkc):
                        k0 = kc * 4
                        nj = min(4, nkt_i - k0)
                        w = nj * P
                        S = pspool.tile([P, 512], f32, tag="S")
                        nc.tensor.matmul(
                            S[:, :w],
                            lhsT=lhsQ,
                            rhs=KT_sb[hb:hb + DK, hs, k0 * P:k0 * P + w],
                            start=True, stop=True,
                        )
                        ST = pspool.tile([P, 512], f32, tag="ST")
                        for j in range(nj):
                            nc.tensor.matmul(
                                ST[:, j * P:(j + 1) * P],
                                lhsT=KT_sb[hb:hb + DK, hs,
                                           (k0 + j) * P:(k0 + j + 1) * P],
                                rhs=QT_sb[hb:hb + DK, hs, qsl],
                                start=True, stop=True,
                            )
                        if causal and k0 <= i < k0 + nj:
                            jd = i - k0
                            dsl = slice(jd * P, (jd + 1) * P)
                            nc.vector.tensor_add(S[:, dsl], S[:, dsl], tri_sb[:])
                            nc.vector.tensor_add(ST[:, dsl], ST[:, dsl], triT_sb[:])

                        if causal:
                            nc.scalar.activation(
                                em[:, k0 * P:k0 * P + w], S[:, :w], AF.Exp,
                                scale=0.125, accum_out=sig[:, kc:kc + 1],
                            )
                            nc.scalar.activation(
                                emT[:, k0:k0 + nj, :], ST[:, :w], AF.Exp,
                                scale=0.125,
                            )
                        else:
                            nc.scalar.activation(
                                em[:, k0 * P:k0 * P + w], S[:, :w], AF.Exp,
                                scale=0.125,
                            )
                            nc.scalar.activation(
                                emT[:, k0:k0 + nj, :], ST[:, :w], AF.Exp,
                                scale=0.125,
                            )
                            # mask + masked row-sum
                            nc.vector.scalar_tensor_tensor(
                                out=em[:, k0 * P:k0 * P + w],
                                in0=em[:, k0 * P:k0 * P + w],
                                scalar=1.0,
                                in1=mrow[:, k0 * P:k0 * P + w],
                                op0=ALU.mult, op1=ALU.mult,
                                accum_out=sig[:, kc:kc + 1],
                            )
                            nc.vector.scalar_tensor_tensor(
                                out=emT[:, k0:k0 + nj, :],
                                in0=emT[:, k0:k0 + nj, :],
                                scalar=1.0,
                                in1=mcol[:, k0:k0 + nj, :],
                                op0=ALU.mult, op1=ALU.mult,
                            )

                    sigt = smpool.tile([P, 1], f32, tag="sigt")
                    nc.vector.reduce_sum(sigt[:], sig[:, :nkc], axis=AX.X)
                    rs = smpool.tile([P, 1], f32, tag="rs")
                    nc.vector.reciprocal(rs[:], sigt[:])

                    # normalized attn tile -> DRAM
                    W = nkt_i * P
                    ao = aopool.tile([P, t_len], f32, tag="ao")
                    nc.vector.tensor_scalar_mul(ao[:, :W], em[:, :W], rs[:])
                    if causal:
                        off = P * P * (i * (i + 1) // 2)
                    else:
                        off = i * P * t_len
                    nc.sync.dma_start(
                        attn_p[h, off:off + P * W].rearrange("(p w) -> p w", p=P),
                        ao[:, :W],
                    )

                    # PV
                    cps = pspool.tile([P, DK], f32, tag="ctx")
                    for kt in range(nkt_i):
                        nc.tensor.matmul(
                            cps[:],
                            lhsT=emT[:, kt, :],
                            rhs=V_sb[:, kt, h * DK:(h + 1) * DK],
                            start=(kt == 0), stop=(kt == nkt_i - 1),
                        )
                    nc.vector.tensor_scalar_mul(
                        ctx_sb[:, h * DK:(h + 1) * DK], cps[:], rs[:]
                    )

                # transpose ctx -> ctxT, then out projection
                ctxT_sb = ctxpool.tile([P, S2, P], f32, tag="ctxt")
                for s in range(S2):
                    tp = pspool.tile([P, P], f32, tag="ST")
                    nc.tensor.transpose(
                        tp[:], ctx_sb[:, s * P:(s + 1) * P], id_sb[:]
                    )
                    nc.vector.tensor_copy(ctxT_sb[:, s, :], tp[:])

                ob = outpool.tile([P, D], f32, tag="ob")
                for nck in range(D // 512):
                    op = pspool.tile([P, 512], f32, tag="S")
                    for s in range(S2):
                        nc.tensor.matmul(
                            op[:],
                            lhsT=ctxT_sb[:, s, :],
                            rhs=wo_sb[:, s, nck * 512:(nck + 1) * 512],
                            start=(s == 0), stop=(s == S2 - 1),
                        )
                    nc.vector.tensor_copy(ob[:, nck * 512:(nck + 1) * 512], op[:])
                nc.sync.dma_start(out_p[qsl, :], ob[:])

    nc.compile()
    return nc


def _get(t_len, causal):
    key = (t_len, causal)
    if key not in _built:
        _built[key] = _build(t_len, causal)
    return _built[key]


def make_in_maps(q_inp, k_inp, v_inp, attn_mask, Wq, bq, Wk, bk, Wv, bv, Wo,
                 causal, t_len):
    """Build the 8 per-core input dicts."""
    ident = np.eye(P, dtype=np.float32)
    r = np.arange(P)
    trim = np.where(r[None, :] > r[:, None], np.float32(NEG), np.float32(0.0))
    in_maps = []
    qT = [np.ascontiguousarray(q_inp[b].T) for b in range(B)]
    kTm = [np.ascontiguousarray(k_inp[b].T) for b in range(B)]
    vTm = [np.ascontiguousarray(v_inp[b].T) for b in range(B)]
    if not causal:
        keep = np.ascontiguousarray((~attn_mask).astype(np.uint8))
        keepT = np.ascontiguousarray(keep.T)
    for c in range(8):
        b, hg = c // 4, c % 4
        cs = slice(hg * DKC, (hg + 1) * DKC)
        m = {
            "qT": qT[b], "kT": kTm[b], "vT": vTm[b],
            "wq": np.ascontiguousarray(Wq[:, cs]),
            "wk": np.ascontiguousarray(Wk[:, cs]),
            "wv": np.ascontiguousarray(Wv[:, cs]),
            "wo": np.ascontiguousarray(Wo[cs, :]),
            "bq2": np.ascontiguousarray(bq[cs].reshape(S2, P).T),
            "bk2": np.ascontiguousarray(bk[cs].reshape(S2, P).T),
            "bv1": np.ascontiguousarray(bv[cs].reshape(1, DKC)),
            "ident": ident,
        }
        if causal:
            m["tri"] = trim
            m["triT"] = np.ascontiguousarray(trim.T)
        else:
            m["mrows"] = keep
            m["mcolsT"] = keepT
        in_maps.append(m)
    return in_maps


def gather(results, bo, causal, t_len):
    NT = t_len // P
    out = np.zeros((B, t_len, D), np.float32)
    attn = np.zeros((B, H, t_len, t_len), np.float32)
    for c in range(8):
        b, hg = c // 4, c % 4
        r = results[c]
        out[b] += r["out_p"]
        pk = r["attn_p"]
        for h in range(HPC):
            off = 0
            for i in range(NT):
                W = ((i + 1) * P) if causal else t_len
                attn[b, hg * HPC + h, i * P:(i + 1) * P, :W] = \
                    pk[h, off:off + P * W].reshape(P, W)
                off += P * W
    out += bo.astype(np.float32)
    return out, attn


def kernel(q_inp, k_inp, v_inp, attn_mask, Wq, bq, Wk, bk, Wv, bv, Wo, bo,
           trace=False):
    from concourse.bass_utils import run_bass_kernel_spmd

    q_inp = np.asarray(q_inp, np.float32)
    k_inp = np.asarray(k_inp, np.float32)
    v_inp = np.asarray(v_inp, np.float32)
    attn_mask = np.asarray(attn_mask, bool)
    Wq, bq = np.asarray(Wq, np.float32), np.asarray(bq, np.float32)
    Wk, bk = np.asarray(Wk, np.float32), np.asarray(bk, np.float32)
    Wv, bv = np.asarray(Wv, np.float32), np.asarray(bv, np.float32)
    Wo, bo = np.asarray(Wo, np.float32), np.asarray(bo, np.float32)

    causal = bool(
        np.array_equal(attn_mask, np.triu(np.ones((T, T), bool), k=1))
    )
    nc = _get(T, causal)
    in_maps = make_in_maps(q_inp, k_inp, v_inp, attn_mask, Wq, bq, Wk, bk,
                           Wv, bv, Wo, causal, T)
    res = run_bass_kernel_spmd(nc, in_maps, core_ids=list(range(8)),
                               trace=trace)
    out, attn = gather(res.results, bo, causal, T)
    kernel.last_results = res
    return out, attn
